# revision 26
# baseline (speedup 1.0000x reference)
"""DeepFM-style embedding reduction kernel for 8 Trainium2 NeuronCores.

Model (reference):
    embf    = emb^T @ x                  # (E,)  E=16, F=2M
    squ     = (emb*emb)^T @ (x*x)        # (E,)
    fm      = 0.5 * (embf^2 - squ)
    h       = relu(relu(embf@w1.T+b1)@w2.T+b2)
    out     = sigmoid(concat(h, fm, embf@w_log.T+b_log) @ w_out.T + b_out)

The F=2M reduction is memory bound (emb is 128MB fp32).  The final output is
a sigmoid deep in its exponential tail (~1.8e-8), so output rel-err ~= abs
logit error; the fm term amplifies embf error by ~2*|embf|*w_out, which rules
out bf16/fp16 input streams (measured 20%/5% output rel err).  int16
fixed-point (scale folded on host) gives 15 mantissa bits -> ~0.3% output
rel err while still halving HBM traffic vs fp32.

Device-side per core (rows split across 8 cores, e-major layout [P, E, CT]):
  - DVE  scalar_tensor_tensor (op0=mult scalar=1, op1=mult) with fused
         accum_out: scaled_bf16 = emb_i16 * x_i16 and the per-partition embf
         partial accumulates in fp32 BEFORE the bf16 output rounding
         (verified on HW: 1e-5 vs exact).  1x DVE mode (~2.19us/row; the
         instruction has no 2x uop).
  - ACT  activation(Square) with fused accum_out -> per-partition squ
         partial (fp32), ~2.2us/row.  scaled values are integer-scaled
         products; bf16 rounding is statistically harmless for the positive
         squ sum.  Both engines run in lockstep, each ~100% busy.
  - No PE, no DVE tensor_reduce (1x-mode, slower than the fused accums).
  - DMA: x (0.5MB) on the ACT HWDGE ring concurrent with emb chunk 0; emb
         (8MB) as 0.5MB per-e-row chunks on the sync ring from a bufs=4
         pool, so chunk k's DMA is gated on compute freeing a buffer
         (compute-paced prefetch instead of 16-way concurrent round-robin).
         Row 0 is computed as two half-row ops (extra accum slot) so compute
         starts as soon as the first halves land.
  Timeline per core: ~8.7us fixed runtime startup (engine code fetch,
  barriers, ACT table load) + ~3.5us first-data + 16 rows x ~2.2us engine
  pipeline + ~3.5us drain/out ~= 53us.
Host: int16 quantize + e-major reshape, final 128-partition + 8-core sum in
float64, tiny MLP head in numpy.
Rejected alternatives (measured/analyzed): bf16/fp16 input streams fail the
2e-2 gate (output is sigmoid(logit~=-17.8), so rel err ~= |dlogit|, and fm
amplifies embf error ~6x -> 20%/5% measured); native-ISA tensor_tensor_reduce
crashes this runtime; gpsimd scalar_tensor_tensor fails to compile; DVE
tensor_reduce and 4-pass fp32 PE matmuls are too slow; PE cannot consume the
e-major stream for exact int16 products without a layout that breaks the
ACT-accum squ path.
"""

import numpy as np

F = 2_000_000
E = 16
P = 128
NCORES = 8
CT = 1954            # free-dim columns per partition per core
S = P * CT           # 250112 rows per core shard (8*S = 2000896 >= F)
EMB_BUFS = 4         # chunk-pool depth: DMA k starts when compute frees k-4
NACC = E + 2         # accum slots: 16 e-rows + extras for row-0/row-15 halves

_cache = {}


def _build_nc():
    from contextlib import ExitStack

    import concourse.bacc as bacc
    import concourse.tile as tile
    from concourse import mybir

    i16 = mybir.dt.int16
    bf16 = mybir.dt.bfloat16
    f32 = mybir.dt.float32
    nc = bacc.Bacc("TRN2", debug=False, num_devices=NCORES)
    x_d = nc.dram_tensor("xq", [P, CT], i16, kind="ExternalInput").ap()
    emb_d = nc.dram_tensor("embq", [P, E * CT], i16, kind="ExternalInput").ap()
    out_d = nc.dram_tensor("out", [P, 2 * NACC], f32, kind="ExternalOutput").ap()

    with ExitStack() as ctx:
        tc = ctx.enter_context(tile.TileContext(nc))
        singles = ctx.enter_context(tc.tile_pool(name="singles", bufs=1))
        embp = ctx.enter_context(tc.tile_pool(name="embp", bufs=EMB_BUFS))
        sclp = ctx.enter_context(tc.tile_pool(name="sclp", bufs=4))
        sqp = ctx.enter_context(tc.tile_pool(name="sqp", bufs=4))

        x_sb = singles.tile([P, CT], i16, name="x_sb")
        pe_t = singles.tile([P, NACC], f32, name="pe_t")   # embf partials
        pq_t = singles.tile([P, NACC], f32, name="pq_t")   # squ partials

        # Head schedule: x halves on the scalar HWDGE ring, e0 halves first
        # on the sync ring -- both streams progress concurrently so the first
        # half-row op starts as soon as x0+e0a land (~10.5us).  Row 0 and
        # row 15 are computed as half-row ops (extra accum slots): row 0 to
        # start compute early, row 15 so the final ACT square overlaps the
        # final DVE half instead of serializing after it.
        H = 976
        nc.scalar.dma_start(out=x_sb[:, 0:H], in_=x_d[:, 0:H])
        nc.scalar.dma_start(out=x_sb[:, H:CT], in_=x_d[:, H:CT])
        e0 = embp.tile([P, CT], i16, tag="embc")
        nc.sync.dma_start(out=e0[:, 0:H], in_=emb_d[:, 0:H])
        nc.sync.dma_start(out=e0[:, H:CT], in_=emb_d[:, H:CT])

        # (erow_ap, x_ap, accum_slot) pieces; row 0 -> slots 0 & E,
        # row 15 -> slots 15 & E+1.
        pieces = [(e0[:, 0:H], x_sb[:, 0:H], 0),
                  (e0[:, H:CT], x_sb[:, H:CT], E)]
        for e in range(1, E):
            et = embp.tile([P, CT], i16, tag="embc")
            nc.sync.dma_start(out=et[:], in_=emb_d[:, e * CT:(e + 1) * CT])
            if e == E - 1:
                pieces.append((et[:, 0:H], x_sb[:, 0:H], e))
                pieces.append((et[:, H:CT], x_sb[:, H:CT], E + 1))
            else:
                pieces.append((et[:], x_sb[:], e))

        for erow, xap, slot in pieces:
            ncol = erow.shape[1]
            scaled = sclp.tile([P, ncol], bf16, tag=f"scaled{ncol}")
            nc.vector.scalar_tensor_tensor(
                out=scaled[:],
                in0=erow,
                scalar=1.0,
                in1=xap,
                op0=mybir.AluOpType.mult,
                op1=mybir.AluOpType.mult,
                accum_out=pe_t[:, slot:slot + 1],
            )
            sq = sqp.tile([P, ncol], bf16, tag=f"sq{ncol}")
            nc.scalar.activation(
                out=sq[:],
                in_=scaled[:],
                func=mybir.ActivationFunctionType.Square,
                accum_out=pq_t[:, slot:slot + 1],
            )

        nc.sync.dma_start(out=out_d[:, 0:NACC], in_=pe_t[:])
        nc.scalar.dma_start(out=out_d[:, NACC:2 * NACC], in_=pq_t[:])

    nc.compile()
    return nc


def _prep_inputs(x, emb):
    """int16-quantize x/emb (scales folded out) and shard e-major per core."""
    x = np.asarray(x, np.float32).reshape(F)
    emb = np.asarray(emb, np.float32).reshape(F, E)
    sx = float(np.max(np.abs(x))) / 32767.0
    se = float(np.max(np.abs(emb))) / 32767.0
    sx = sx if sx > 0 else 1.0
    se = se if se > 0 else 1.0
    xq = np.clip(np.rint(x * (1.0 / sx)), -32767, 32767).astype(np.int16)
    eq = np.clip(np.rint(emb * (1.0 / se)), -32767, 32767).astype(np.int16)
    total = NCORES * S
    if total > F:
        xq = np.concatenate([xq, np.zeros(total - F, np.int16)])
        eq = np.concatenate([eq, np.zeros((total - F, E), np.int16)])
    in_maps = []
    for k in range(NCORES):
        xs = xq[k * S:(k + 1) * S].reshape(P, CT)
        es = eq[k * S:(k + 1) * S].reshape(P, CT, E).transpose(0, 2, 1)
        in_maps.append({
            "xq": np.ascontiguousarray(xs),
            "embq": np.ascontiguousarray(es).reshape(P, E * CT),
        })
    return in_maps, sx, se


def _ensure_ntff_hook():
    """The agent image's antenv lacks axon_hooks; provide it + register the
    ctypes NTFF profiling hook against the axon PJRT .so (trace-only path)."""
    import sys
    import types

    try:
        from antenv.axon_hooks import get_axon_ntff_profile_hook  # noqa: F401
        return
    except ImportError:
        pass
    mod = types.ModuleType("antenv.axon_hooks")
    _h = [None]
    mod.set_axon_ntff_profile_hook = lambda h: _h.__setitem__(0, h)
    mod.get_axon_ntff_profile_hook = lambda: _h[0]
    sys.modules["antenv.axon_hooks"] = mod
    try:
        import antenv
        antenv.axon_hooks = mod
    except ImportError:
        pass

    import contextlib
    import ctypes

    so_path = "/opt/axon/libaxon_pjrt.so"
    try:
        lib = ctypes.CDLL(so_path)
    except OSError:
        return
    if not hasattr(lib, "axon_start_nrt_profile"):
        return
    lib.axon_start_nrt_profile.argtypes = [ctypes.POINTER(ctypes.c_int64),
                                           ctypes.c_size_t]
    lib.axon_start_nrt_profile.restype = ctypes.c_int64
    lib.axon_stop_nrt_profile.argtypes = [ctypes.c_char_p]
    lib.axon_stop_nrt_profile.restype = ctypes.c_int64

    @contextlib.contextmanager
    def _hook(output_dir, device_ids):
        import jax
        jax.devices()
        if device_ids:
            ids = (ctypes.c_int64 * len(device_ids))(*device_ids)
            rc = lib.axon_start_nrt_profile(ids, len(device_ids))
        else:
            rc = lib.axon_start_nrt_profile(None, 0)
        if rc != 0:
            raise RuntimeError(f"axon_start_nrt_profile rc={rc}")
        try:
            yield
        finally:
            n = lib.axon_stop_nrt_profile(str(output_dir).encode())
            print(f"ntff profile: {n} file(s) -> {output_dir}")

    mod.set_axon_ntff_profile_hook(_hook)


def _run_device(x, emb, trace=False):
    from concourse.bass_utils import run_bass_kernel_spmd

    if trace:
        _ensure_ntff_hook()
    if "nc" not in _cache:
        _cache["nc"] = _build_nc()
    nc = _cache["nc"]
    in_maps, sx, se = _prep_inputs(x, emb)
    res = run_bass_kernel_spmd(nc, in_maps, core_ids=list(range(NCORES)),
                               trace=trace)
    parts = np.stack([r["out"].astype(np.float64) for r in res.results])
    totals = parts.sum(axis=(0, 1))              # [2*NACC]
    pe = totals[:NACC].copy()
    pq = totals[NACC:].copy()
    pe[0] += pe[E]
    pq[0] += pq[E]
    pe[E - 1] += pe[E + 1]
    pq[E - 1] += pq[E + 1]
    embf = (pe[:E] * (sx * se)).astype(np.float32)
    squ = (pq[:E] * (sx * se) ** 2).astype(np.float32)
    return embf, squ, res


def _mlp_head(embf, squ, w_log, b_log, w1, b1, w2, b2, w_out, b_out):
    embf = embf.astype(np.float32)
    squ = squ.astype(np.float32)
    logistic = embf @ w_log.T + b_log                       # (1,)
    fm = 0.5 * (embf * embf - squ)                          # (E,)
    h = np.maximum(embf @ w1.T + b1, 0.0)
    h = np.maximum(h @ w2.T + b2, 0.0)
    concat = np.concatenate([h, fm, logistic]).astype(np.float32)
    logit = concat @ w_out.T + b_out
    return (1.0 / (1.0 + np.exp(-logit))).astype(np.float32)


def kernel(x, emb, w_log, b_log, w1, b1, w2, b2, w_out, b_out, _trace=False):
    x = np.asarray(x, np.float32)
    emb = np.asarray(emb, np.float32)
    embf, squ, res = _run_device(x, emb, trace=_trace)
    out = _mlp_head(embf, squ,
                    np.asarray(w_log, np.float32), np.asarray(b_log, np.float32),
                    np.asarray(w1, np.float32), np.asarray(b1, np.float32),
                    np.asarray(w2, np.float32), np.asarray(b2, np.float32),
                    np.asarray(w_out, np.float32), np.asarray(b_out, np.float32))
    if _trace:
        kernel.last_results = res
    return out
